# revision 13
# baseline (speedup 1.0000x reference)
"""GAttNHP model as a Bass/Tile kernel on 8 Trainium2 NeuronCores.

Strategy: pure data-parallel over batch (B=16 -> 2 batches/core, no
collectives).  bf16 matmuls accumulating in fp32 PSUM; the dominant
intensity head runs fp8e4 with DoubleRow (2 weights/cell).  Sequence dim
padded 511 -> 512; pad rows/cols are host-zeroed in all scatter/gather
matrices so they never reach real outputs.

Device pipeline, emitted as a complete chain PER BATCH so batch 1's
latency-bound front half overlaps batch 0's compute-bound intensity
matmuls:
  1. AttNHP encoder, 2 layers, activations kept in transposed [d, t]
     layout (plus a natural [t, d] copy for the group scatter).
     Causal softmax in s^T layout: exp (no max-subtract, scores are
     tiny), triangular mask on the diagonal block, column sums via an
     appended ones-column on v, 1/colsum via ACT exp(-ln(x)) (the DVE
     reciprocal on a 1-partition row costs ~2.8us), normalization via a
     rank-1 broadcast matmul.
  2. Group scatter-mean as a matmul against a host-built one-hot
     matrix (64 batch-local segments on partitions 0..63).
  3. Tiny group transformer block (attn + FFN + 2 layernorms; rstd via
     ln/exp so the whole kernel uses ONE ACT table set --
     natural_log_exp_and_others; see _pin_act_tables).
  4. Gather back + merge Linear computed directly in transposed layout
     enh^T[e, r], written as fp8.
  5. Intensity head logits[r, n] = enh @ int_w: fp8 DoubleRow matmuls
     (K=256/instruction) into 2-bank [128,1024] PSUM strips; softplus
     as exp (un-scaling the x128 weight scale for free) then ln(x+1).

Constant-per-batch subject/relation embedding columns of `feats` are
folded on the host into a gp bias and a per-batch merge-bias row (seeded
into PSUM with a K=1 matmul).

NB: within one PSUM accumulation group all matmuls must overlap in PE
row-groups (partition ranges of the contraction) -- disjoint row-groups
execute concurrently in different sub-arrays and race on the PSUM
accumulate, which faults the exec unit (NRT_EXEC_UNIT_UNRECOVERABLE).
"""

import os

import numpy as np
import ml_dtypes

bf16 = ml_dtypes.bfloat16
f8 = ml_dtypes.float8_e4m3
INTW_SCALE = 128.0

N_ENTITY = 8000
N_REL = 100
N_GROUPS = 64
HIDDEN = 256
D_MODEL = 256
N_LAYERS = 2
N_HEADS = 4
GP = 64
GH = 2
D_TOTAL = D_MODEL * N_LAYERS          # 512
D_FEAT = D_TOTAL + 2 * HIDDEN         # 1024
B, L = 16, 512
Lh = L - 1                            # 511
NCORES = 8
BPC = B // NCORES                     # 2 batches per core
T = 512                               # padded seq length
NT = T // 128                         # 4 t-tiles per batch
R = BPC * T                           # 1024 rows per core
NSEG = N_GROUPS                       # 64 batch-local segments
NE_PAD = 8192
NPAIR = 8                             # 1024-col strips (last covers 832)

LAST_EXEC_NS = None
LAST_RESULTS = None
_CACHED = {}


def _time_enc(t, d=D_MODEL):
    i = np.arange(d // 2)
    freqs = np.exp(-np.log(10000.0) * (2.0 * i / d)).astype(np.float32)
    ang = t[..., None].astype(np.float32) * freqs
    return np.concatenate([np.sin(ang), np.cos(ang)], axis=-1).astype(np.float32)


def _pack_T(a):
    # [512, 256] natural -> [128, 2, 512] transposed tiles (d = c*128+p)
    return np.ascontiguousarray(a.T.reshape(2, 128, T).transpose(1, 0, 2))


def _pack_N(a):
    # [512, 256] natural -> [128, 4, 256] natural tiles (t = m*128+p)
    return np.ascontiguousarray(a.reshape(NT, 128, D_MODEL).transpose(1, 0, 2))


def _wpack(w):
    # [256, 256] -> [128, 2, 256]  (rows d = c*128+p)
    return np.ascontiguousarray(w.reshape(2, 128, D_MODEL).transpose(1, 0, 2))


def prep_inputs(inputs):
    """Returns (in_maps per core, notes dict)."""
    f32 = np.float32
    subs = np.asarray(inputs["subs"])
    marks = np.asarray(inputs["marks"])
    objs = np.asarray(inputs["objs"])
    times = np.asarray(inputs["times"], f32)
    dt = np.asarray(inputs["dt"], f32)
    mask = np.asarray(inputs["mask"])
    group_map = np.asarray(inputs["group_map"])
    g = lambda k: np.asarray(inputs[k], f32)
    obj_embed = g("obj_embed")
    core_Wq, core_Wk, core_Wv, core_Wo = (
        g("core_Wq"), g("core_Wk"), g("core_Wv"), g("core_Wo"))
    sub_embed, rel_embed = g("sub_embed"), g("rel_embed")
    gp_w, gp_b = g("gp_w"), g("gp_b")
    ga_in_w, ga_in_b = g("ga_in_w"), g("ga_in_b")
    ga_out_w, ga_out_b = g("ga_out_w"), g("ga_out_b")
    ffn_w1, ffn_b1, ffn_w2, ffn_b2 = g("ffn_w1"), g("ffn_b1"), g("ffn_w2"), g("ffn_b2")
    n1_w, n1_b, n2_w, n2_b = g("n1_w"), g("n1_b"), g("n2_w"), g("n2_b")
    mg_w, mg_b = g("mg_w"), g("mg_b")
    int_w, int_b = g("int_w"), g("int_b")

    shared = {}
    shared["wq"] = np.stack([_wpack(core_Wq[l] / np.sqrt(64.0))
                             for l in range(N_LAYERS)]).astype(bf16)
    shared["wk"] = np.stack([_wpack(core_Wk[l]) for l in range(N_LAYERS)]).astype(bf16)
    shared["wv"] = np.stack([_wpack(core_Wv[l]) for l in range(N_LAYERS)]).astype(bf16)
    shared["wo"] = np.stack([_wpack(core_Wo[l]) for l in range(N_LAYERS)]).astype(bf16)
    shared["gpw"] = np.ascontiguousarray(
        gp_w[:D_TOTAL].reshape(4, 128, GP).transpose(1, 0, 2)).astype(bf16)
    gain = ga_in_w.copy()
    gainb = ga_in_b.copy().reshape(3, GP).T.copy()   # [64, 3] columns q/k/v
    gain[:, :GP] /= np.sqrt(32.0)
    gainb[:, 0] /= np.sqrt(32.0)
    shared["gain"] = gain.astype(bf16)
    shared["gainb"] = gainb.astype(f32)
    shared["gaout"] = ga_out_w.astype(bf16)
    shared["gaoutb"] = ga_out_b.reshape(GP, 1).astype(f32)
    shared["fw1"] = ffn_w1.astype(bf16)
    shared["fw2"] = ffn_w2.astype(bf16)
    shared["fb1"] = ffn_b1.reshape(1, GP).astype(bf16)
    shared["fb2"] = ffn_b2.reshape(1, GP).astype(bf16)
    shared["lnw1"] = np.tile(n1_w, (NSEG, 1)).astype(f32)
    shared["lnb1"] = np.tile(n1_b, (NSEG, 1)).astype(f32)
    shared["lnw2"] = np.tile(n2_w, (NSEG, 1)).astype(f32)
    shared["lnb2"] = np.tile(n2_b, (NSEG, 1)).astype(f32)
    shared["mgw1"] = np.ascontiguousarray(
        mg_w[:D_TOTAL].reshape(4, 128, 8, 128).transpose(1, 2, 0, 3)).astype(bf16)
    shared["mgw2"] = np.ascontiguousarray(
        mg_w[D_FEAT:D_FEAT + GP].reshape(GP, 8, 128)).astype(bf16)
    wpad = np.zeros((D_FEAT, NE_PAD), np.float32)
    wpad[:, :N_ENTITY] = int_w * INTW_SCALE
    shared["intw"] = np.ascontiguousarray(
        wpad.reshape(8, 128, NPAIR, 1024).transpose(2, 0, 1, 3)).astype(f8)
    tri = (np.arange(128)[None, :] >= np.arange(128)[:, None])
    shared["tri"] = tri.astype(bf16)
    with_intb = not np.allclose(int_b, 0.0)
    if with_intb:
        ibp = np.zeros((1, NE_PAD), np.float32)
        ibp[0, :N_ENTITY] = int_b * INTW_SCALE
        shared["intb"] = ibp.astype(bf16)

    in_maps = []
    for core in range(NCORES):
        m = dict(shared)
        xT = np.zeros((BPC, 128, 2, T), np.float32)
        c0T = np.zeros((BPC, 128, 2, T), np.float32)
        c0n = np.zeros((BPC, 128, NT, D_MODEL), np.float32)
        mscT = np.zeros((128, BPC * NT, NSEG), np.float32)  # [p, rt, seg]
        mga = np.zeros((NSEG, BPC, T), np.float32)          # [seg, b, t]
        gpbias = np.zeros((NSEG, BPC, GP), np.float32)
        mgc = np.zeros((1, BPC, D_FEAT), np.float32)
        for b in range(BPC):
            gb = core * BPC + b
            hist = objs[gb, :Lh]
            x_nat = np.zeros((T, D_MODEL), np.float32)
            x_nat[:Lh] = (obj_embed[hist] + _time_enc(times[gb, :Lh])
                          + _time_enc(dt[gb, :Lh]))
            cur0 = np.zeros((T, D_MODEL), np.float32)
            cur0[:Lh] = _time_enc(times[gb, 1:])
            xT[b] = _pack_T(x_nat)
            c0T[b] = _pack_T(cur0)
            c0n[b] = _pack_N(cur0)

            gids = group_map[subs[gb] * N_REL + marks[gb]][:Lh]
            fm = mask[gb, :Lh].astype(np.float32)
            cnt = np.bincount(gids, weights=fm, minlength=NSEG)
            ts = np.arange(Lh)
            mga[gids, b, ts] = fm
            msc = np.zeros((T, NSEG), np.float32)            # [t, seg]
            msc[ts, gids] = fm / np.maximum(cnt, 1.0)[gids]
            mscT[:, NT * b:NT * b + NT, :] = msc.reshape(
                NT, 128, NSEG).transpose(1, 0, 2)
            sr = np.concatenate([sub_embed[subs[gb, 0]], rel_embed[marks[gb, 0]]])
            nz = (cnt > 0).astype(np.float32)
            gpbias[:, b, :] = (nz[:, None] * (sr @ gp_w[D_TOTAL:D_FEAT])[None, :]
                               + gp_b[None, :])
            mgc[0, b] = sr @ mg_w[D_TOTAL:D_FEAT] + mg_b
        m["xT"] = xT.astype(bf16)
        m["c0T"] = c0T.astype(bf16)
        m["c0n"] = c0n.astype(bf16)
        m["mscT"] = mscT.astype(bf16)
        m["mgath"] = mga.astype(bf16)
        m["gpbias"] = gpbias
        m["mgc"] = mgc.astype(bf16)
        in_maps.append(m)
    return in_maps, {"with_intb": with_intb}


def _pin_act_tables():
    # bacc assigns each InstActivation a table set greedily, which makes the
    # intensity epilogue (exp then ln per tile) alternate between
    # exp_and_others and natural_log -> one ~1.3us ACT_TABLE_LOAD per tile
    # (334us total!).  Empty every set except natural_log_exp_and_others
    # (which contains Exp/Ln/Copy/Identity/Square -- everything we use) so
    # the chooser is forced onto one set; positional set ids are preserved.
    import concourse.bacc as bacc
    from concourse import hw_specs
    if getattr(bacc.get_activation_tables, "_pinned", False):
        return
    orig = hw_specs.get_activation_tables
    KEEP = "natural_log_exp_and_others"

    def pinned(arch):
        t = dict(orig(arch))
        return {k: (v if k == KEEP else set()) for k, v in t.items()}

    pinned._pinned = True
    bacc.get_activation_tables = pinned


def build_nc(use_softplus=True, with_intb=False, debug_stop=99):
    import concourse.bacc as bacc
    import concourse.mybir as mybir
    import concourse.tile as tile
    from concourse import masks as cmasks
    _pin_act_tables()

    dtb = mybir.dt.bfloat16
    dtf = mybir.dt.float32
    dt8 = mybir.dt.float8e4
    AF = mybir.ActivationFunctionType
    ALU = mybir.AluOpType
    AX = mybir.AxisListType
    DR = mybir.MatmulPerfMode.DoubleRow

    nc = bacc.Bacc()

    def din(name, shape, dt=dtb):
        return nc.dram_tensor(name, shape, dt, kind="ExternalInput")

    xT_d = din("xT", [BPC, 128, 2, T])
    c0T_d = din("c0T", [BPC, 128, 2, T])
    c0n_d = din("c0n", [BPC, 128, NT, D_MODEL])
    wq_d = din("wq", [N_LAYERS, 128, 2, D_MODEL])
    wk_d = din("wk", [N_LAYERS, 128, 2, D_MODEL])
    wv_d = din("wv", [N_LAYERS, 128, 2, D_MODEL])
    wo_d = din("wo", [N_LAYERS, 128, 2, D_MODEL])
    mscT_d = din("mscT", [128, BPC * NT, NSEG])
    mgath_d = din("mgath", [NSEG, BPC, T])
    gpw_d = din("gpw", [128, 4, GP])
    gpbias_d = din("gpbias", [NSEG, BPC, GP], mybir.dt.float32)
    gain_d = din("gain", [GP, 3 * GP])
    gainb_d = din("gainb", [GP, 3], mybir.dt.float32)
    gaout_d = din("gaout", [GP, GP])
    gaoutb_d = din("gaoutb", [GP, 1], mybir.dt.float32)
    fw1_d = din("fw1", [GP, GP])
    fw2_d = din("fw2", [GP, GP])
    fb1_d = din("fb1", [1, GP])
    fb2_d = din("fb2", [1, GP])
    lnw1_d = din("lnw1", [NSEG, GP], mybir.dt.float32)
    lnb1_d = din("lnb1", [NSEG, GP], mybir.dt.float32)
    lnw2_d = din("lnw2", [NSEG, GP], mybir.dt.float32)
    lnb2_d = din("lnb2", [NSEG, GP], mybir.dt.float32)
    mgw1_d = din("mgw1", [128, 8, 4, 128])
    mgw2_d = din("mgw2", [GP, 8, 128])
    mgc_d = din("mgc", [1, BPC, D_FEAT])
    intw_d = din("intw", [NPAIR, 8, 128, 1024], dt8)
    tri_d = din("tri", [128, 128])
    if with_intb:
        intb_d = din("intb", [1, NE_PAD])
    out_d = nc.dram_tensor("out", [R, N_ENTITY], mybir.dt.float32,
                           kind="ExternalOutput")

    with tile.TileContext(nc) as tc:
        with (
            tc.tile_pool(name="persist", bufs=1) as pp,
            tc.tile_pool(name="work", bufs=2) as wp,
            tc.tile_pool(name="acts", bufs=3) as ap,
            tc.tile_pool(name="intw", bufs=2) as iwp,
            tc.tile_pool(name="outp", bufs=4) as op,
            tc.tile_pool(name="ps_i", bufs=2, space="PSUM") as ps_i,
            tc.tile_pool(name="ps_a", bufs=2, space="PSUM") as ps_a,
            tc.tile_pool(name="ps_o", bufs=1, space="PSUM") as ps_o,
            tc.tile_pool(name="ps_s", bufs=1, space="PSUM") as ps_s,
        ):
            def pt(shape, tag, dt=dtb):
                return pp.tile(shape, dt, tag=tag, name=tag)

            def dma(dst, src):
                nc.sync.dma_start(dst, src)

            # ---- constants in ----
            xT = [pt([128, 2, T], f"xT{b}") for b in range(BPC)]
            c0T = [pt([128, 2, T], f"c0T{b}") for b in range(BPC)]
            c0n = [pt([128, NT, D_MODEL], f"c0n{b}") for b in range(BPC)]
            for b in range(BPC):
                dma(xT[b][:], xT_d[b])
                dma(c0T[b][:], c0T_d[b])
                dma(c0n[b][:], c0n_d[b])
            wq = [pt([128, 2, D_MODEL], f"wq{l}") for l in range(N_LAYERS)]
            wk = [pt([128, 2, D_MODEL], f"wk{l}") for l in range(N_LAYERS)]
            wo = [pt([128, 2, D_MODEL], f"wo{l}") for l in range(N_LAYERS)]
            wv = [pt([128, 2, D_MODEL], f"wv{l}") for l in range(N_LAYERS)]
            for l in range(N_LAYERS):
                dma(wq[l][:], wq_d[l])
                dma(wk[l][:], wk_d[l])
                dma(wv[l][:], wv_d[l])
                dma(wo[l][:], wo_d[l])
            tri_s = pt([128, 128], "tri")
            dma(tri_s[:], tri_d[:])
            mscT_s = pt([128, BPC * NT, NSEG], "mscT")
            dma(mscT_s[:], mscT_d[:])
            mgath_s = pt([NSEG, BPC, T], "mgath")
            dma(mgath_s[:], mgath_d[:])
            gpw_s = pt([128, 4, GP], "gpw")
            dma(gpw_s[:], gpw_d[:])
            gpbias_s = pt([NSEG, BPC, GP], "gpbias", dtf)
            dma(gpbias_s[:], gpbias_d[:])
            gain_s = pt([GP, 3 * GP], "gain")
            dma(gain_s[:], gain_d[:])
            gainb_s = pt([GP, 3], "gainb", dtf)
            dma(gainb_s[:], gainb_d[:])
            gaout_s = pt([GP, GP], "gaout")
            dma(gaout_s[:], gaout_d[:])
            gaoutb_s = pt([GP, 1], "gaoutb", dtf)
            dma(gaoutb_s[:], gaoutb_d[:])
            fw1_s = pt([GP, GP], "fw1")
            dma(fw1_s[:], fw1_d[:])
            fw2_s = pt([GP, GP], "fw2")
            dma(fw2_s[:], fw2_d[:])
            fb1_s = pt([1, GP], "fb1")
            dma(fb1_s[:], fb1_d[:])
            fb2_s = pt([1, GP], "fb2")
            dma(fb2_s[:], fb2_d[:])
            ln_s = {}
            for nm, d in [("lnw1", lnw1_d), ("lnb1", lnb1_d),
                          ("lnw2", lnw2_d), ("lnb2", lnb2_d)]:
                ln_s[nm] = pt([NSEG, GP], nm, dtf)
                dma(ln_s[nm][:], d[:])
            mgw1_s = pt([128, 8, 4, 128], "mgw1")
            dma(mgw1_s[:], mgw1_d[:])
            mgw2_s = pt([GP, 8, 128], "mgw2")
            dma(mgw2_s[:], mgw2_d[:])
            mgc_s = pt([1, BPC, D_FEAT], "mgc")
            dma(mgc_s[:], mgc_d[:])
            if with_intb:
                intb_s = pt([1, NE_PAD], "intb")
                dma(intb_s[:], intb_d[:])

            eps_s = pt([NSEG, 1], "eps", dtf)
            nc.gpsimd.memset(eps_s[:], 1e-5)
            ident = pt([128, 128], "ident")
            cmasks.make_identity(nc, ident[:])
            ones_r = pt([1, T], "ones_r")
            nc.gpsimd.memset(ones_r[:], 1.0)

            def layernorm(xin, wtile, btile, outf, outb):
                P = xin.shape[0]
                s1 = wp.tile([P, 1], dtf, tag="lns", name="lns")
                nc.vector.reduce_sum(s1[:], xin[:], axis=AX.X)
                mu = wp.tile([P, 1], dtf, tag="lnm", name="lnm")
                nc.vector.tensor_scalar_mul(mu[:], s1[:], 1.0 / GP)
                xc = wp.tile([P, GP], dtf, tag="lnxc", name="lnxc")
                nc.vector.tensor_scalar(xc[:], xin[:], mu[:], None,
                                        op0=ALU.subtract)
                sq = wp.tile([P, GP], dtf, tag="lnsq", name="lnsq")
                vs = wp.tile([P, 1], dtf, tag="lnvs", name="lnvs")
                nc.scalar.activation(sq[:], xc[:], AF.Square, accum_out=vs[:])
                lnv = wp.tile([P, 1], dtf, tag="lnlv", name="lnlv")
                nc.scalar.activation(lnv[:], vs[:], AF.Ln, scale=1.0 / GP,
                                     bias=eps_s[:P])
                rstd = wp.tile([P, 1], dtf, tag="lnrs", name="lnrs")
                nc.scalar.activation(rstd[:], lnv[:], AF.Exp, scale=-0.5)
                nc.vector.scalar_tensor_tensor(
                    outf[:], xc[:], rstd[:], wtile[:], op0=ALU.mult, op1=ALU.mult)
                nc.vector.tensor_add(outf[:], outf[:], btile[:])
                nc.vector.tensor_copy(outb[:], outf[:])

            # ======== full pipeline, one complete chain per batch ========
            for b in range(BPC):
                # ---------------- AttNHP encoder ----------------
                curT, curn = c0T[b], c0n[b]
                encT = [None] * N_LAYERS
                encn = [None] * N_LAYERS
                for l in range(N_LAYERS):
                    qT = ap.tile([128, 2, T], dtb, tag="qT")
                    kT = ap.tile([128, 2, T], dtb, tag="kT")
                    for c2 in range(2):
                        psq = ps_a.tile([128, T], dtf, tag="a", name="psq")
                        for c in range(2):
                            nc.tensor.matmul(
                                psq[:], wq[l][:, c, 128 * c2:128 * c2 + 128],
                                curT[:, c, :], start=(c == 0), stop=(c == 1))
                        nc.vector.tensor_copy(qT[:, c2, :], psq[:])
                        psk = ps_a.tile([128, T], dtf, tag="a", name="psk")
                        for c in range(2):
                            nc.tensor.matmul(
                                psk[:], wk[l][:, c, 128 * c2:128 * c2 + 128],
                                xT[b][:, c, :], start=(c == 0), stop=(c == 1))
                        nc.vector.tensor_copy(kT[:, c2, :], psk[:])
                    vcat = []
                    for m in range(NT):
                        psv = ps_s.tile([128, D_MODEL], dtf, tag="s", name="psv")
                        for c in range(2):
                            nc.tensor.matmul(
                                psv[:], xT[b][:, c, 128 * m:128 * m + 128],
                                wv[l][:, c, :], start=(c == 0), stop=(c == 1))
                        vc = ap.tile([128, N_HEADS, 65], dtb, tag=f"vcat{m}",
                                     name=f"vcat{m}")
                        nc.vector.tensor_copy(
                            vc[:, :, 0:64],
                            psv[:].rearrange("p (h e) -> p h e", h=N_HEADS))
                        nc.vector.memset(vc[:, :, 64:65], 1.0)
                        vcat.append(vc)
                    oT = ap.tile([128, 2, T], dtb, tag="oT")
                    for h in range(N_HEADS):
                        bp, hc = 64 * (h % 2), h // 2
                        po = ps_o.tile([65, T], dtf, tag="o", name="po")
                        for j in range(NT):
                            q0 = 128 * j
                            nq = T - q0
                            pss = ps_a.tile([128, T], dtf, tag="a", name="pss")
                            nc.tensor.matmul(
                                pss[:, :nq],
                                kT[bp:bp + 64, hc, q0:q0 + 128],
                                qT[bp:bp + 64, hc, q0:T])
                            aT = ap.tile([128, T], dtb, tag="aT")
                            nc.scalar.activation(aT[:, :nq], pss[:, :nq], AF.Exp)
                            nc.vector.tensor_mul(
                                aT[:, 0:128], aT[:, 0:128], tri_s[:])
                            nc.tensor.matmul(
                                po[:, q0:T], vcat[j][:, h, :], aT[:, :nq],
                                start=(j == 0), stop=(j == NT - 1),
                                skip_group_check=True)
                        rsb = wp.tile([1, T], dtf, tag="rsb")
                        nc.scalar.activation(rsb[:], po[64:65, :], AF.Ln)
                        rbf = wp.tile([1, T], dtb, tag="rbf")
                        nc.scalar.activation(rbf[:], rsb[:], AF.Exp, scale=-1.0)
                        oraw = wp.tile([64, T], dtb, tag="oraw")
                        nc.vector.tensor_copy(oraw[:], po[0:64, :])
                        pb = ps_s.tile([64, T], dtf, tag="s", name="pb")
                        nc.tensor.matmul(pb[:], ones_r[0:1, 0:64], rbf[:])
                        nc.vector.tensor_mul(oT[bp:bp + 64, hc, :], oraw[:], pb[:])
                    eT = pt([128, 2, T], f"encT{l}{b}")
                    en = pt([128, NT, D_MODEL], f"encn{l}{b}")
                    for c2 in range(2):
                        psp = ps_a.tile([128, T], dtf, tag="a", name="psp")
                        for c in range(2):
                            nc.tensor.matmul(
                                psp[:], wo[l][:, c, 128 * c2:128 * c2 + 128],
                                oT[:, c, :], start=(c == 0), stop=(c == 1))
                        nc.vector.tensor_add(eT[:, c2, :], curT[:, c2, :], psp[:])
                    for m in range(NT):
                        psn = ps_s.tile([128, D_MODEL], dtf, tag="s", name="psn")
                        for c in range(2):
                            nc.tensor.matmul(
                                psn[:], oT[:, c, 128 * m:128 * m + 128],
                                wo[l][:, c, :], start=(c == 0), stop=(c == 1))
                        nc.vector.tensor_add(en[:, m, :], curn[:, m, :], psn[:])
                    encT[l], encn[l] = eT, en
                    curT, curn = eT, en
                if debug_stop < 3:
                    continue

                # ------- group scatter A^T[d, seg] (batch-local) -------
                AT_s = wp.tile([128, 4, NSEG], dtb, tag="AT", name="AT")
                for dt_i in range(4):
                    l, c2 = dt_i // 2, dt_i % 2
                    pa = ps_a.tile([128, NSEG], dtf, tag="a", name="pa")
                    for m in range(NT):
                        nc.tensor.matmul(
                            pa[:], encn[l][:, m, 128 * c2:128 * c2 + 128],
                            mscT_s[:, NT * b + m, :],
                            start=(m == 0), stop=(m == NT - 1))
                    nc.vector.tensor_copy(AT_s[:, dt_i, :], pa[:])

                # ---------------- group block ([64, *]) ----------------
                pg = ps_s.tile([NSEG, GP], dtf, tag="s", name="pg")
                for dt_i in range(4):
                    nc.tensor.matmul(pg[:], AT_s[:, dt_i, :], gpw_s[:, dt_i, :],
                                     start=(dt_i == 0), stop=(dt_i == 3))
                gp_f = wp.tile([NSEG, GP], dtf, tag="gp_f")
                nc.vector.tensor_add(gp_f[:], pg[:], gpbias_s[:, b, :])
                gp_bf = wp.tile([NSEG, GP], dtb, tag="gp_bf")
                nc.vector.tensor_copy(gp_bf[:], gp_f[:])
                ptr = ps_s.tile([GP, NSEG], dtb, tag="s", name="ptr")
                nc.tensor.transpose(ptr[:], gp_bf[:], ident[0:NSEG, 0:NSEG])
                gpT = wp.tile([GP, NSEG], dtb, tag="gpT")
                nc.vector.tensor_copy(gpT[:], ptr[:])
                qkvT = []
                for i in range(3):
                    pq = ps_s.tile([GP, NSEG], dtf, tag="s", name="pq")
                    nc.tensor.matmul(pq[:], gain_s[:, GP * i:GP * i + GP], gpT[:])
                    tq = wp.tile([GP, NSEG], dtb, tag=f"qkvT{i}", name=f"qkvT{i}")
                    nc.vector.tensor_scalar(
                        tq[:], pq[:], gainb_s[:, i:i + 1], None, op0=ALU.add)
                    qkvT.append(tq)
                vc2 = wp.tile([GP, GH, 33], dtb, tag="vc2", name="vc2")
                for h in range(GH):
                    pv = ps_s.tile([GP, 32], dtb, tag="s", name="pv")
                    nc.tensor.transpose(
                        pv[:], qkvT[2][32 * h:32 * h + 32, :],
                        ident[32 * h:32 * h + 32, 32 * h:32 * h + 32])
                    nc.vector.tensor_copy(vc2[:, h, 0:32], pv[:])
                nc.vector.memset(vc2[:, :, 32:33], 1.0)
                oT2 = wp.tile([GP, NSEG], dtb, tag="oT2")
                for h in range(GH):
                    ps1 = ps_s.tile([64, 64], dtf, tag="s", name="ps1")
                    nc.tensor.matmul(
                        ps1[:], qkvT[1][32 * h:32 * h + 32, :],
                        qkvT[0][32 * h:32 * h + 32, :])
                    a2 = wp.tile([64, 64], dtb, tag="a2")
                    nc.scalar.activation(a2[:], ps1[:], AF.Exp)
                    po2 = ps_s.tile([33, 64], dtf, tag="s", name="po2")
                    nc.tensor.matmul(po2[:], vc2[:, h, :], a2[:])
                    r2 = wp.tile([1, 64], dtf, tag="r2")
                    nc.scalar.activation(r2[:], po2[32:33, :], AF.Ln)
                    r2b = wp.tile([1, 64], dtb, tag="r2b")
                    nc.scalar.activation(r2b[:], r2[:], AF.Exp, scale=-1.0)
                    pb2 = ps_s.tile([32, 64], dtf, tag="s", name="pb2")
                    nc.tensor.matmul(pb2[:], ones_r[0:1, 0:32], r2b[:])
                    orw2 = wp.tile([32, 64], dtb, tag="orw2")
                    nc.vector.tensor_copy(orw2[:], po2[0:32, :])
                    nc.vector.tensor_mul(oT2[32 * h:32 * h + 32, :], orw2[:],
                                         pb2[:])
                pga = ps_s.tile([GP, NSEG], dtf, tag="s", name="pga")
                nc.tensor.matmul(pga[:], gaout_s[:], oT2[:])
                t1 = wp.tile([GP, NSEG], dtb, tag="t1")
                nc.vector.scalar_tensor_tensor(
                    t1[:], pga[:], gaoutb_s[:, 0:1], gpT[:],
                    op0=ALU.add, op1=ALU.add)
                px1 = ps_s.tile([NSEG, GP], dtb, tag="s", name="px1")
                nc.tensor.transpose(px1[:], t1[:], ident[0:GP, 0:GP])
                x1 = wp.tile([NSEG, GP], dtf, tag="x1")
                nc.vector.tensor_copy(x1[:], px1[:])
                gn_f = wp.tile([NSEG, GP], dtf, tag="gn_f")
                gn_b = wp.tile([NSEG, GP], dtb, tag="gn_b")
                layernorm(x1, ln_s["lnw1"], ln_s["lnb1"], gn_f, gn_b)
                pgt = ps_s.tile([GP, NSEG], dtb, tag="s", name="pgt")
                nc.tensor.transpose(pgt[:], gn_b[:], ident[0:NSEG, 0:NSEG])
                gnT = wp.tile([GP, NSEG], dtb, tag="gnT")
                nc.vector.tensor_copy(gnT[:], pgt[:])
                ph1 = ps_s.tile([NSEG, GP], dtf, tag="s", name="ph1")
                nc.tensor.matmul(ph1[:], ones_r[0:1, 0:NSEG], fb1_s[:],
                                 start=True, stop=False)
                nc.tensor.matmul(ph1[:], gnT[:], fw1_s[:], start=False, stop=True)
                h1b = wp.tile([NSEG, GP], dtb, tag="h1b")
                nc.vector.tensor_scalar_max(h1b[:], ph1[:], 0.0)
                ph1t = ps_s.tile([GP, NSEG], dtb, tag="s", name="ph1t")
                nc.tensor.transpose(ph1t[:], h1b[:], ident[0:NSEG, 0:NSEG])
                h1T = wp.tile([GP, NSEG], dtb, tag="h1T")
                nc.vector.tensor_copy(h1T[:], ph1t[:])
                ph2 = ps_s.tile([NSEG, GP], dtf, tag="s", name="ph2")
                nc.tensor.matmul(ph2[:], ones_r[0:1, 0:NSEG], fb2_s[:],
                                 start=True, stop=False)
                nc.tensor.matmul(ph2[:], h1T[:], fw2_s[:], start=False, stop=True)
                x2 = wp.tile([NSEG, GP], dtf, tag="x2")
                nc.vector.tensor_add(x2[:], ph2[:], gn_f[:])
                go_f = wp.tile([NSEG, GP], dtf, tag="go_f")
                go_b = wp.tile([NSEG, GP], dtb, tag="go_b")
                layernorm(x2, ln_s["lnw2"], ln_s["lnb2"], go_f, go_b)

                # ---------------- gather^T ----------------
                gathT = wp.tile([GP, NT, 128], dtb, tag="gathT", name="gathT")
                for m in range(NT):
                    pgh = ps_s.tile([GP, 128], dtf, tag="s", name="pgh")
                    nc.tensor.matmul(pgh[:], go_b[:],
                                     mgath_s[:, b, 128 * m:128 * m + 128])
                    nc.vector.tensor_copy(gathT[:, m, :], pgh[:])
                if debug_stop < 4:
                    continue

                # ----------- merge linear -> enh^T (fp8) -----------
                enhT = pt([128, 8, T], f"enhT{b}", dt8)
                for et in range(8):
                    pe = ps_a.tile([128, T], dtf, tag="a", name="pe")
                    nc.tensor.matmul(
                        pe[:], mgc_s[0:1, b, 128 * et:128 * et + 128], ones_r[:],
                        start=True, stop=False, skip_group_check=True)
                    for dt_i in range(4):
                        l, c2 = dt_i // 2, dt_i % 2
                        nc.tensor.matmul(
                            pe[:], mgw1_s[:, et, dt_i, :], encT[l][:, c2, :],
                            start=False, stop=False, skip_group_check=True)
                    nc.tensor.matmul(
                        pe[:], mgw2_s[:, et, :],
                        gathT[:].rearrange("p a c -> p (a c)"),
                        start=False, stop=True, skip_group_check=True)
                    nc.vector.tensor_copy(enhT[:, et, :], pe[:])
                if debug_stop < 5:
                    continue

                # ----------- intensity head (fp8 DoubleRow) -----------
                for nbp in range(NPAIR):
                    nn2 = 1024 if nbp < NPAIR - 1 else N_ENTITY - 1024 * (NPAIR - 1)
                    wna = iwp.tile([128, 8, 1024], dt8, tag="intw", name="wna")
                    nc.sync.dma_start(
                        wna[:], intw_d[nbp].rearrange("c p j -> p c j"))
                    for m in range(NT):
                        pi = ps_i.tile([128, 1024], dtf, tag="i", name="pi")
                        for half in range(2):
                            h0 = 512 * half
                            ncol = min(512, nn2 - h0)
                            if with_intb:
                                nc.tensor.matmul(
                                    pi[:, h0:h0 + ncol], ones_r[0:1, 0:128],
                                    intb_s[0:1, 1024 * nbp + h0:
                                           1024 * nbp + h0 + ncol],
                                    start=True, stop=False, skip_group_check=True)
                            for c in range(4):
                                nc.tensor.matmul(
                                    pi[:, h0:h0 + ncol],
                                    enhT[:, 2 * c:2 * c + 2,
                                         128 * m:128 * m + 128],
                                    wna[:, 2 * c:2 * c + 2, h0:h0 + ncol],
                                    perf_mode=DR,
                                    start=(c == 0 and not with_intb),
                                    stop=(c == 3), skip_group_check=True)
                        ot = op.tile([128, 1024], dtf, tag="out")
                        # softplus = ln(1 + exp(x)); both funcs live in the
                        # natural_log_exp_and_others table set (no switch).
                        nc.scalar.activation(ot[:, :nn2], pi[:, :nn2], AF.Exp,
                                             scale=1.0 / INTW_SCALE)
                        if use_softplus:
                            nc.scalar.activation(
                                ot[:, :nn2], ot[:, :nn2], AF.Ln, bias=1.0)
                        nc.sync.dma_start(
                            out_d[T * b + 128 * m:T * b + 128 * m + 128,
                                  1024 * nbp:1024 * nbp + nn2],
                            ot[:, :nn2])

    nc.compile()
    return nc


def _get_nc(use_softplus, with_intb):
    key = (use_softplus, with_intb)
    if key not in _CACHED:
        _CACHED[key] = build_nc(use_softplus, with_intb)
    return _CACHED[key]


def _install_ntff_hook():
    """Best-effort: register the axon NTFF profile hook so trace=True works."""
    import sys, types
    if "antenv.axon_hooks" in sys.modules:
        return
    try:
        import antenv  # noqa
        from trn_agent_boot.trn_boot import _ntff_profile_via_ctypes
        mod = types.ModuleType("antenv.axon_hooks")
        hook = [_ntff_profile_via_ctypes("/opt/axon/libaxon_pjrt.so")]
        mod.set_axon_ntff_profile_hook = lambda h: hook.__setitem__(0, h)
        mod.get_axon_ntff_profile_hook = lambda: hook[0]
        sys.modules["antenv.axon_hooks"] = mod
    except Exception:
        pass


def kernel(**inputs):
    global LAST_EXEC_NS, LAST_RESULTS
    from concourse.bass_utils import run_bass_kernel_spmd

    in_maps, notes = prep_inputs(inputs)
    nc = _get_nc(True, notes["with_intb"])
    trace = bool(os.environ.get("BASS_TRACE"))
    if trace:
        _install_ntff_hook()
    res = run_bass_kernel_spmd(
        nc, in_maps, core_ids=list(range(NCORES)), trace=trace)
    LAST_RESULTS = res
    LAST_EXEC_NS = res.exec_time_ns
    out = np.empty((B, Lh, N_ENTITY), np.float32)
    for core in range(NCORES):
        o = res.results[core]["out"]
        for b in range(BPC):
            out[core * BPC + b] = o[T * b:T * b + Lh, :]
    return out
